# revision 23
# baseline (speedup 1.0000x reference)
import sys

sys.path.insert(0, "/opt/trn_rl_repo")

from contextlib import ExitStack

import ml_dtypes
import numpy as np

import concourse.bass as bass
import concourse.mybir as mybir
import concourse.tile as tile
from concourse import bacc, bass_utils

N, OBS, ENC, ACT, K = 16384, 512, 512, 64, 8
ALPHA = 1.0
NCORES = 8
P = 128
NJ = 5  # DoubleRow slab pairs: (x0a,x0b),(x0c,x0d),(x1a,x1b),(x1c,x1d),(u,0)
F32 = mybir.dt.float32
BF16 = mybir.dt.bfloat16
FP8 = mybir.dt.float8e4
DR = mybir.MatmulPerfMode.DoubleRow
NP8 = ml_dtypes.float8_e4m3
NWARM = 14


def _solve_assign(pat, needs):
    # slots: NCORES copies of each nonzero size in pat; find per-expert slot
    # multisets covering `needs` (ordered desc). DFS over waste-ordered options.
    from itertools import product as iproduct

    sizes = {}
    for s in pat:
        if s > 0:
            sizes[s] = sizes.get(s, 0) + NCORES
    svals = sorted(sizes, reverse=True)
    scnt = [sizes[s] for s in svals]
    budget = [0]

    def dfs(i, avail):
        budget[0] += 1
        if budget[0] > 20000:
            return None
        if i == len(needs):
            return []
        need = needs[i]
        if sum(a * s for a, s in zip(avail, svals)) < sum(needs[i:]):
            return None
        opts = []
        maxn = [min(a, -(-need // s) if s else 0) for a, s in zip(avail, svals)]
        for combo in iproduct(*[range(m + 1) for m in maxn]):
            cap = sum(n * s for n, s in zip(combo, svals))
            if cap < need:
                continue
            # drop combos with a removable slot
            if any(n > 0 and cap - s >= need for n, s in zip(combo, svals)):
                continue
            opts.append((cap - need, combo))
        opts.sort()
        for _, combo in opts:
            rest = dfs(i + 1, [a - n for a, n in zip(avail, combo)])
            if rest is not None:
                got = []
                for n, s in zip(combo, svals):
                    got += [s] * n
                return [got] + rest
        return None

    return dfs(0, scnt)


def _plan(tile_counts):
    # Find per-core slot pattern (a,b,c) and an assignment of the 8*3 slots to
    # experts so each expert k gets slots with total capacity >= tile_counts[k].
    # Returns (pattern, assign) where assign[k] = list of slot sizes granted.
    total = int(sum(tile_counts))
    t_sorted = sorted(range(K), key=lambda k: -tile_counts[k])
    base = -(-total // NCORES)
    best = None
    for t_pc in range(base, base + 3):
        pats = []
        for a in range(-(-t_pc // 3), t_pc + 1):
            for b in range(0, min(a, t_pc - a) + 1):
                c = t_pc - a - b
                if c <= b and c >= 0:
                    pats.append((a, b, c))
        for pat in pats:
            assign = _solve_assign(pat, [int(tile_counts[k]) for k in t_sorted])
            if assign is not None:
                best = (pat, {k: assign[i] for i, k in enumerate(t_sorted)})
                break
        if best is not None:
            break
    if best is None:
        # fallback: expert-sharded, one slot per core
        t_max = max(1, int(max(tile_counts)))
        return (t_max, 0, 0), {k: [t_max] for k in range(K)}
    return best


def build_nc(pattern):
    # Per-core: slots of `pattern` tiles, each slot has its own folded expert
    # matrices. Per 128-row tile: 5 fp8 DoubleRow matmuls accumulate
    #   e = x0@(4M) + u@(8B) - x1@(4W^T)   (pairwise scales cancel)
    # into one PSUM bank; ACT squares from PSUM, DVE reduces, Pool sums at end.
    slots = [s for s in pattern if s > 0]
    t_pc = sum(slots)
    nc = bacc.Bacc("TRN2", target_bir_lowering=False)
    zt = nc.declare_dram_parameter("zt", [P, t_pc, 2 * NJ, P], FP8, isOutput=False)
    # slot 0's matrices split in two transfers (earlier first-tile start);
    # later slots ship as one tensor each
    da0 = nc.declare_dram_parameter("da0", [P, 4, ENC], FP8, isOutput=False)
    db0 = nc.declare_dram_parameter("db0", [P, 6, ENC], FP8, isOutput=False)
    dss = [
        nc.declare_dram_parameter(f"ds{s}", [P, 2 * NJ, ENC], FP8, isOutput=False)
        for s in range(1, len(slots))
    ]
    loss = nc.declare_dram_parameter("loss_out", [1, 1], F32, isOutput=True)

    # z DMA groups: small first for early start, then 4-wide
    groups, off = [], 0
    for w in [1, 2, 3]:
        if off < t_pc:
            w = min(w, t_pc - off)
            groups.append((off, w))
            off += w
    while off < t_pc:
        w = min(4, t_pc - off)
        groups.append((off, w))
        off += w
    t2g = {}
    for gi, (goff, w) in enumerate(groups):
        for t in range(goff, goff + w):
            t2g[t] = gi
    # slot of each tile
    t2s, soff = {}, [0]
    for si, s in enumerate(slots):
        for t in range(soff[-1], soff[-1] + s):
            t2s[t] = si
        soff.append(soff[-1] + s)

    with tile.TileContext(nc) as tc, ExitStack() as ctx:
        const = ctx.enter_context(tc.tile_pool(name="const", bufs=1))
        stream = ctx.enter_context(tc.tile_pool(name="stream", bufs=len(groups)))
        dwork = ctx.enter_context(tc.tile_pool(name="dwork", bufs=4))
        psum = ctx.enter_context(tc.tile_pool(name="psum", bufs=8, space="PSUM"))

        # PE warmup on zeroed scratch: ramps the p-state while DMAs run.
        wz = const.tile([P, 2, P], FP8)
        wd = const.tile([P, 2, ENC], FP8)
        nc.gpsimd.memset(wz[:], 0)
        nc.gpsimd.memset(wd[:], 0)
        for _ in range(NWARM):
            pw = psum.tile([P, ENC], F32, name="pw", tag="ps")
            nc.tensor.matmul(pw[:], wz[:], wd[:], start=True, stop=True, perf_mode=DR)

        da0_sb = const.tile([P, 4, ENC], FP8)
        db0_sb = const.tile([P, 6, ENC], FP8)
        ds_sb = [
            const.tile([P, 2 * NJ, ENC], FP8, name=f"dssb{s}")
            for s in range(1, len(slots))
        ]
        acc = const.tile([P, t_pc], F32)

        def d_ap(si, j):
            if si == 0:
                return (
                    da0_sb[:, 2 * j : 2 * j + 2, :]
                    if j < 2
                    else db0_sb[:, 2 * (j - 2) : 2 * (j - 2) + 2, :]
                )
            return ds_sb[si - 1][:, 2 * j : 2 * j + 2, :]

        # DMA issue order: slot-0 matrices, early z groups, then interleave
        # remaining matrices ahead of the z tiles that need them.
        issued_d = {0}
        zs = [None] * len(groups)

        def issue_d(si):
            if si not in issued_d:
                issued_d.add(si)
                nc.sync.dma_start(ds_sb[si - 1][:], dss[si - 1][:])

        nc.sync.dma_start(da0_sb[:], da0[:])
        for gi, (goff, w) in enumerate(groups):
            # make sure matrices for slots touched by the NEXT group are in flight
            zg = stream.tile([P, 4, 2 * NJ, P], FP8, name="zg")
            nc.sync.dma_start(zg[:, 0:w], zt[:, goff : goff + w])
            zs[gi] = zg
            if gi == 0:
                nc.sync.dma_start(db0_sb[:], db0[:])
            if gi + 1 < len(groups):
                ngoff, nw = groups[gi + 1]
                for t in range(ngoff, ngoff + nw):
                    issue_d(t2s[t])

        for t in range(t_pc):
            gi = t2g[t]
            i = t - groups[gi][0]
            si = t2s[t]
            ps = psum.tile([P, ENC], F32, name="ps", tag="ps")
            for j in range(NJ):
                nc.tensor.matmul(
                    ps[:],
                    zs[gi][:, i, 2 * j : 2 * j + 2, :],
                    d_ap(si, j),
                    start=(j == 0),
                    stop=(j == NJ - 1),
                    perf_mode=DR,
                )
            sj = dwork.tile([P, ENC], BF16, name="sj")
            nc.scalar.activation(
                sj[:],
                ps[:],
                mybir.ActivationFunctionType.Square,
                accum_out=acc[:, t : t + 1],
            )

        out_sb = const.tile([1, 1], F32)
        nc.gpsimd.tensor_reduce(
            out_sb[:], acc[:], axis=mybir.AxisListType.XYZWC, op=mybir.AluOpType.add
        )
        nc.sync.dma_start(loss[:], out_sb[:])

    nc.finalize()
    return nc


_NC_CACHE = {}
_LAST_KEY = None


def _get_nc(pattern=None):
    key = _LAST_KEY if pattern is None else pattern
    if key not in _NC_CACHE:
        _NC_CACHE[key] = build_nc(key)
    return _NC_CACHE[key]


def make_in_maps(X1, X0, U, W_enc, A_all, B_rest, C_w, C_b):
    global _LAST_KEY
    X1, X0, U = np.asarray(X1), np.asarray(X0), np.asarray(U)
    W_enc, A_all, B_rest = np.asarray(W_enc), np.asarray(A_all), np.asarray(B_rest)
    C_w, C_b = np.asarray(C_w), np.asarray(C_b)

    # f64 router on host: argmax(X0 @ W_enc.T @ C_w.T + C_b) per row
    m = (C_w.astype(np.float64) @ W_enc.astype(np.float64)).T  # [OBS, K]
    inds = np.argmax(X0.astype(np.float64) @ m + C_b.astype(np.float64), axis=1)
    counts = np.bincount(inds, minlength=K)
    tile_counts = [-(-int(c) // P) for c in counts]
    pattern, assign = _plan(tile_counts)
    _LAST_KEY = pattern
    slots = [s for s in pattern if s > 0]
    nslot = len(slots)
    t_pc = sum(slots)

    # quantize data once (pair scales cancel against the matrices)
    x0q = (X0 * 0.25).astype(NP8)
    x1q = (X1 * 0.25).astype(NP8)
    uq = (U * 0.125).astype(NP8)

    wT = W_enc.T.astype(np.float32)  # [OBS, ENC]
    wn4 = -4.0 * wT
    B0 = np.eye(ENC, dtype=np.float32)[:ACT]
    Ball = np.concatenate([B0[None], B_rest.astype(np.float32)], axis=0)

    d8 = {}
    for c in range(K):
        m4 = 4.0 * (wT @ A_all[c].T.astype(np.float32))  # [OBS, ENC]
        dslab = np.zeros((2 * NJ, P, ENC), np.float32)
        dslab[0:4] = m4.reshape(4, P, ENC)
        dslab[4:8] = wn4.reshape(4, P, ENC)
        dslab[8, :ACT, :] = 8.0 * Ball[c]
        d8[c] = dslab.astype(NP8)
    dzero = np.zeros((2 * NJ, P, ENC), NP8)

    # distribute each expert's slot grants to (core, slot_index) positions:
    # free positions per slot size, one (a,b,c) triple per core
    free = {si: list(range(NCORES)) for si in range(nslot)}
    # map slot size -> slot indices having that size (sizes can repeat)
    size2si = {}
    for si, s in enumerate(slots):
        size2si.setdefault(s, []).append(si)
    core_slots = [[None] * nslot for _ in range(NCORES)]  # (expert, n_tiles_here)
    for k in sorted(range(K), key=lambda k: -tile_counts[k]):
        rem = tile_counts[k]
        for s in sorted(assign[k], reverse=True):
            placed = False
            for si in size2si[s]:
                if free[si]:
                    c = free[si].pop(0)
                    take = min(rem, s)
                    core_slots[c][si] = (k, take)
                    rem -= take
                    placed = True
                    break
            assert placed, "slot placement failed"
    # rows per expert, consumed in order
    rowptr = {k: 0 for k in range(K)}
    rowlist = {k: np.nonzero(inds == k)[0] for k in range(K)}

    in_maps = []
    soff = np.cumsum([0] + slots)
    for c in range(NCORES):
        zz = np.zeros((2 * NJ, P, t_pc * P), NP8)  # [slab, comp, n]
        im = {}
        for si in range(nslot):
            ent = core_slots[c][si]
            dk = dzero
            if ent is not None:
                k, ntile_k = ent
                p0 = rowptr[k]
                rows = rowlist[k][p0 : p0 + ntile_k * P]
                rowptr[k] = p0 + len(rows)
                nr = len(rows)
                n0 = int(soff[si]) * P
                zz[0:4, :, n0 : n0 + nr] = x0q[rows].T.reshape(4, P, nr)
                zz[4:8, :, n0 : n0 + nr] = x1q[rows].T.reshape(4, P, nr)
                zz[8, :ACT, n0 : n0 + nr] = uq[rows].T
                dk = d8[k]
            if si == 0:
                im["da0"] = np.ascontiguousarray(dk[0:4].transpose(1, 0, 2))
                im["db0"] = np.ascontiguousarray(dk[4:10].transpose(1, 0, 2))
            else:
                im[f"ds{si}"] = np.ascontiguousarray(dk.transpose(1, 0, 2))
        im["zt"] = np.ascontiguousarray(
            zz.reshape(2 * NJ, P, t_pc, P).transpose(1, 2, 0, 3)
        )  # [p, t, slab, r]
        in_maps.append(im)
    return in_maps


def kernel(X1, X0, U, W_enc, A_all, B_rest, C_w, C_b):
    in_maps = make_in_maps(X1, X0, U, W_enc, A_all, B_rest, C_w, C_b)
    nc = _get_nc()
    res = bass_utils.run_bass_kernel_spmd(nc, in_maps, list(range(NCORES)))
    total = sum(float(r["loss_out"][0, 0]) for r in res.results)
    return np.float32(ALPHA * total / (ENC * N))
